# revision 1
# baseline (speedup 1.0000x reference)
"""AntisymmetricRNN Trainium2 kernel — 8-core data-parallel over batch.

Math (per reference):
    mask = strictly-lower-tri; w_r = v_r * mask; A = w_r - w_r.T
    step:  h' = h + (1/TAU) * tanh( tanh(h) @ A + b_r - GAMMA*h )
           x_pred = tanh(h') @ w_o.T + b_o;   err_t = x_pred - x_t

Design (v2):
  * batch 256 sharded 8 ways (32 per core); recurrence local per core.
  * state layout "h-major": [128 partitions = h%128, free = (h//128, b)] so
    the recurrent matmul output (z^T accumulated per h-out tile into PSUM)
    lands in exactly the state layout -> zero transposes anywhere.
  * per step: 64 bf16 matmuls, lhsT = A 128x128 tiles (stationary, FWL),
    rhs = tanh(h) [128, 32] slices (moving).  z, h, t are split into G=4
    chunks of 64 free elems so DVE/ACT elementwise pipelines against PE.
  * t = b_r - GAMMA*h is precomputed off the critical chain; chain per chunk
    is TT(z+t) -> ACT tanh -> STT(h += 0.1 u) -> ACT tanh -> bf16 th.
  * tanh(h) tiles rotate through a 4-deep slot ring; every 4 steps the
    output projection runs inline as column-tiled matmuls (one slot per PE
    column group, w_o^T as the moving operand) so x_pred lands
    (step,batch)-major and the x / err DMAs are fully contiguous 128KB
    blocks.  No DRAM history round-trip, no separate phase B.
  * fully unrolled (no hardware loops).
"""

import numpy as np
import ml_dtypes
from contextlib import ExitStack

import concourse.bass as bass
import concourse.tile as tile
from concourse import mybir
from concourse.bass_utils import run_bass_kernel_spmd

# ---------------- problem constants (hardcoded per spec) ----------------
S, B, D, H = 512, 256, 256, 1024
NCORES = 8
BS = B // NCORES                  # 32 batch per core
TAU, GAMMA = 10.0, 0.1
INV_TAU = 1.0 / TAU
KT = H // 128                     # 8 contraction tiles
MT = H // 128                     # 8 output tiles
G = 4                             # elementwise chunks per step
CW = (MT // G) * BS               # chunk width in free elems (64)
MPQ = MT // G                     # m-tiles per chunk (2)
NSLOT = 4                         # tanh(h) slot ring depth = proj batch steps

TRACE = False                     # set True from test harness for profiling
LAST_RESULTS = None               # BassKernelResults stash for the harness

_BUILT = None


def _split_multi_waits(nc, max_waits: int = 1):
    """The walrus build here supports one sync-wait slot on CTRL-encoded
    instructions; split any multi-wait instruction's extra waits into a chain
    of preceding single-wait NOPs on the same engine (identical semantics)."""
    for fn in nc.m.functions:
        for bb in fn.blocks:
            new_insts = []
            for inst in bb.instructions:
                si = inst.sync_info
                if si is not None and len(si.on_wait) > max_waits:
                    waits = list(si.on_wait)
                    for w in waits[:-max_waits]:
                        nop = mybir.InstNoOp(
                            name=nc.get_next_instruction_name(), ins=[], outs=[])
                        nop.engine = inst.engine
                        nop.sync_info = mybir.SyncInfo(on_wait=[w], on_update=[])
                        nc.register_instruction(nop)
                        new_insts.append(nop)
                    si.on_wait = waits[-max_waits:]
                new_insts.append(inst)
            bb.instructions = new_insts


def _build_bass():
    nc = bass.Bass("TRN2", target_bir_lowering=False, debug=False,
                   num_devices=NCORES)
    dt = mybir.dt
    f32, bf16 = dt.float32, dt.bfloat16

    A_d = nc.dram_tensor("A", [128, KT * MT * 128], bf16, kind="ExternalInput").ap()
    Wo_d = nc.dram_tensor("Wo", [128, KT * D], bf16, kind="ExternalInput").ap()
    Br_d = nc.dram_tensor("Br", [128, MT * BS], f32, kind="ExternalInput").ap()
    h0_d = nc.dram_tensor("h0", [128, MT * BS], f32, kind="ExternalInput").ap()
    th0_d = nc.dram_tensor("th0", [128, MT * BS], bf16, kind="ExternalInput").ap()
    x_d = nc.dram_tensor("x", [S, BS, D], f32, kind="ExternalInput").ap()
    err_d = nc.dram_tensor("err", [S, BS, D], f32, kind="ExternalOutput").ap()

    Tanh = mybir.ActivationFunctionType.Tanh
    MUL, ADD, SUB = (mybir.AluOpType.mult, mybir.AluOpType.add,
                     mybir.AluOpType.subtract)

    # [S,BS,D] viewed as [S/4, (4*BS)=128, D]: one contiguous 128KB block per
    # 4-step group, partition = (step_low, b).
    x_g = x_d.rearrange("(g s) b d -> g (s b) d", s=NSLOT)
    e_g = err_d.rearrange("(g s) b d -> g (s b) d", s=NSLOT)

    with tile.TileContext(nc) as tc, ExitStack() as ctx:
        const = ctx.enter_context(tc.tile_pool(name="const", bufs=1))
        state = ctx.enter_context(tc.tile_pool(name="state", bufs=1))
        scratch = ctx.enter_context(tc.tile_pool(name="scratch", bufs=3))
        zpool = ctx.enter_context(tc.tile_pool(name="zps", bufs=1, space="PSUM"))
        xppool = ctx.enter_context(tc.tile_pool(name="xpps", bufs=2, space="PSUM"))
        xtp = ctx.enter_context(tc.tile_pool(name="xt", bufs=3))
        etp = ctx.enter_context(tc.tile_pool(name="et", bufs=3))

        A_sb = const.tile([128, KT * MT * 128], bf16, tag="A", name="A_sb")
        Wo_sb = const.tile([128, KT * D], bf16, tag="Wo", name="Wo_sb")
        Br_sb = const.tile([128, MT * BS], f32, tag="Br", name="Br_sb")
        nc.sync.dma_start(A_sb[:], A_d[:])
        nc.sync.dma_start(Wo_sb[:], Wo_d[:])
        nc.sync.dma_start(Br_sb[:], Br_d[:])

        hT = [state.tile([128, CW], f32, tag=f"h{q}", name=f"hT{q}")
              for q in range(G)]
        # tanh(h) slot ring: TH[slot][q]; slot j%NSLOT holds th after step j.
        TH = [[state.tile([128, CW], bf16, tag=f"TH{s}_{q}", name=f"TH{s}_{q}")
               for q in range(G)] for s in range(NSLOT)]
        zT = [zpool.tile([128, CW], f32, tag=f"z{q}", name=f"zT{q}")
              for q in range(G)]
        zeros = const.tile([128, 128], bf16, tag="zeros", name="zeros")
        nc.vector.memset(zeros[:], 0.0)
        for q in range(G):
            sl = slice(q * CW, (q + 1) * CW)
            nc.sync.dma_start(hT[q][:], h0_d[:, sl])
            nc.sync.dma_start(TH[NSLOT - 1][q][:], th0_d[:, sl])
        # Prime PSUM has_written bits with a zero matmul, then pre-write
        # t = b_r - GAMMA*h into each z bank; every step's matmuls accumulate
        # on top (start=False), so the bank holds s = th@A + b_r - GAMMA*h
        # when the k-loop finishes.
        def emit_prewrite(q):
            nc.vector.scalar_tensor_tensor(
                zT[q][:], hT[q][:], -GAMMA, Br_sb[:, q * CW:(q + 1) * CW],
                MUL, ADD)
        for q in range(G):
            nc.tensor.matmul(zT[q][:], lhsT=zeros[:], rhs=zeros[:, :CW],
                             start=True, stop=True)
            emit_prewrite(q)

        # M1 emission order: defer m0/m1's k=6,7 tiles so the previous step's
        # last tanh chunk has time to land before its first consumer.
        M1_ORDER = (
            [(m, k) for m in (0, 1) for k in range(6)]
            + [(0, 6), (0, 7), (1, 6), (1, 7)]
            + [(m, k) for m in range(2, MT) for k in range(KT)]
        )

        def emit_step(j):
            rd, wr = (j - 1) % NSLOT, j % NSLOT
            # M1: z^T[m] += A[k,m]^T @ th[k]   (64 bf16 matmuls)
            for m, k in M1_ORDER:
                q, mo = divmod(m, MPQ)
                zs = zT[q][:, mo * BS:(mo + 1) * BS]
                kq, ko = divmod(k, MPQ)
                nc.tensor.matmul(
                    zs,
                    lhsT=A_sb[:, (k * MT + m) * 128:(k * MT + m + 1) * 128],
                    rhs=TH[rd][kq][:, ko * BS:(ko + 1) * BS],
                    start=False, stop=(k == KT - 1), skip_group_check=True)
            for q in range(G):
                u_t = scratch.tile([128, CW], f32, tag="u", name="u_t")
                nc.scalar.activation(u_t[:], zT[q][:], Tanh)       # u = tanh(s)
                nc.vector.scalar_tensor_tensor(                    # h += u/TAU
                    hT[q][:], u_t[:], INV_TAU, hT[q][:], MUL, ADD)
                nc.scalar.activation(TH[wr][q][:], hT[q][:], Tanh)  # th = tanh(h)
                if j < S - 1:
                    emit_prewrite(q)                               # t for step j+1

        def emit_proj(g):
            # steps 4g..4g+3 -> slots 0..3; x_pred^T via column-tiled matmuls:
            # xp[32*jj+b, d] = sum_h th_slot_jj[h, b] * w_o[d, h]
            xp = xppool.tile([128, D], f32, tag="xp", name="xp")
            for k in range(KT):
                kq, ko = divmod(k, MPQ)
                for jj in range(NSLOT):
                    nc.tensor.matmul(
                        xp[32 * jj:32 * (jj + 1), :],
                        lhsT=TH[jj][kq][:, ko * BS:(ko + 1) * BS],
                        rhs=Wo_sb[:, k * D:(k + 1) * D],
                        start=(k == 0), stop=(k == KT - 1),
                        tile_position=(0, 32 * jj))
            xt = xtp.tile([128, D], f32, tag="xt", name="xt")
            nc.sync.dma_start(xt[:], x_g[g])
            et = etp.tile([128, D], f32, tag="et", name="et")
            nc.vector.scalar_tensor_tensor(                        # xp - (x - b_o)
                et[:], xp[:], 0.0, xt[:], ADD, SUB)
            nc.sync.dma_start(e_g[g], et[:])

        for j in range(S):
            emit_step(j)
            if j % NSLOT == NSLOT - 1:
                emit_proj(j // NSLOT)

    _split_multi_waits(nc)
    return nc


def _host_prep(x, h_init, v_r, b_r, w_o, b_o):
    """Build per-core input maps (all layout work in numpy)."""
    x = np.asarray(x, np.float32)
    h_init = np.asarray(h_init, np.float32)
    v_r = np.asarray(v_r, np.float32)
    b_r = np.asarray(b_r, np.float32)
    w_o = np.asarray(w_o, np.float32)
    b_o = np.asarray(b_o, np.float32)

    mask = np.tril(np.ones((H, H), np.float32), -1)
    w_r = v_r * mask
    A = w_r - w_r.T                                           # [H, H]
    # A_sb[p, (k*MT+m)*128 + c] = A[k*128+p, m*128+c]
    A_sb = np.ascontiguousarray(
        A.reshape(KT, 128, MT, 128).transpose(1, 0, 2, 3).reshape(128, KT * MT * 128)
    ).astype(ml_dtypes.bfloat16)
    # Wo_sb[p, k*D + d] = w_o[d, k*128+p]   (w_o^T tiles, moving operand)
    Wo_sb = np.ascontiguousarray(
        w_o.T.reshape(KT, 128, D).transpose(1, 0, 2).reshape(128, KT * D)
    ).astype(ml_dtypes.bfloat16)
    # Br[p, m*BS+b] = b_r[m*128+p]
    Br = np.ascontiguousarray(
        np.broadcast_to(b_r.reshape(MT, 128, 1).transpose(1, 0, 2), (128, MT, BS))
    ).reshape(128, MT * BS).astype(np.float32)

    in_maps = []
    for c in range(NCORES):
        hc = h_init[c * BS:(c + 1) * BS]                       # [BS, H]
        h0 = np.ascontiguousarray(
            hc.reshape(BS, MT, 128).transpose(2, 1, 0)         # [128, MT, BS]
        ).reshape(128, MT * BS).astype(np.float32)
        th0 = np.tanh(h0)
        in_maps.append({
            "A": A_sb, "Wo": Wo_sb, "Br": Br,
            "h0": h0, "th0": th0.astype(ml_dtypes.bfloat16),
            "x": np.ascontiguousarray(x[:, c * BS:(c + 1) * BS, :] - b_o),
        })
    return in_maps


def kernel(x, h_init, v_r, b_r, w_o, b_o):
    global _BUILT, LAST_RESULTS
    if _BUILT is None:
        _BUILT = _build_bass()
    nc = _BUILT
    in_maps = _host_prep(x, h_init, v_r, b_r, w_o, b_o)
    res = run_bass_kernel_spmd(nc, in_maps, core_ids=list(range(NCORES)),
                               trace=TRACE)
    LAST_RESULTS = res
    out = np.empty((S, B, D), np.float32)
    for c in range(NCORES):
        out[:, c * BS:(c + 1) * BS, :] = np.asarray(res.results[c]["err"])
    return out

